# revision 6
# baseline (speedup 1.0000x reference)
"""Trainium2 Bass kernel for SimCLR-style contrastive loss (B=8192, D=512).

Math (matches reference):
    f_norm = f / ||f||
    sim    = f_norm @ f_norm.T / T
    lse_i  = logsumexp_{j != i} sim_ij
    pos_i  = sim[i, (i + B/2) mod B]
    loss   = mean_i(lse_i - pos_i)

Device strategy (8 cores, data parallel over rows):
  - Host passes each core a row-rotated transposed bf16 copy [D, B]
    (for the gram matmul) plus the same rotation row-major [B, D]
    (for norms), so one SPMD program works for every core: core c's
    rotated copy starts at global row c*R, its own rows occupy local
    columns [0, R), and row r's positive partner sits at local column
    r + B/2.
  - Norms from the row-major copy: square + free-axis reduce on DVE
    (all bf16 SBUF, 2x mode) giving ssq in [128, B/128] partition-major
    layout, so the ln -> exp(-0.5*ln + ln 4) chain on ACT costs ~64
    cycles per 16 columns instead of a [1, B] pass.  The x16 on the
    gram keeps fp8 features clear of the e4m3 subnormal range and is
    compensated in the final exp scale.  inv goes through a DRAM
    round-trip to reshape [128, B/128] -> [1, B], then GpSimd
    partition_broadcast -> [128, B].
  - Features scaled in place on DVE (bf16 2x), then cast bf16 -> fp8e4
    by gpsimd-issued SWDGE DMAs (the only engine whose DMA casts)
    into DoubleRow layout [128, 2, B].
  - Main gram matmul in fp8e4 MatmulPerfMode.DoubleRow: one
    instruction contracts K=256, so D=512 streams each output column
    only twice.  PSUM groups are 2048 wide (2 bufs x 4 banks = all 8,
    norms no longer need PSUM) to amortize ACT per-instruction cost.
  - Each [128 x 2048] PSUM group gets: diagonal killed with a -1e30*I
    accumulating bf16 matmul, partner diagonal extracted via
    tensor_mul against (-1/(16T))*I + row reduce, then fused
    exp+row-sum on ACT (exp((16cos)/(16T) - 1/T), in place in PSUM).
  - Per-row output y_i = ln(sum_i) - pos_cos_i/T returned as [R] f32;
    host computes loss = 1/T + mean(y).
"""

import functools
import sys

sys.path.insert(0, "/opt/trn_rl_repo")

import ml_dtypes
import numpy as np

import concourse.bass as bass
import concourse.mybir as mybir
import concourse.tile as tile
from concourse import bacc
from concourse.bass_utils import run_bass_kernel_spmd
from concourse.tile import add_dep_helper

B = 8192
D = 512
NCORES = 8
R = B // NCORES  # rows per core
TEMP = 0.07
INV_T = 1.0 / TEMP
FSCALE = 16.0  # features carry 4/norm each -> gram = 16*cos
LN_FS_HALF = float(np.log(4.0))

F32 = mybir.dt.float32
BF16 = mybir.dt.bfloat16
FP8 = mybir.dt.float8e4
AF = mybir.ActivationFunctionType
ALU = mybir.AluOpType
DR = mybir.MatmulPerfMode.DoubleRow


@functools.lru_cache(maxsize=None)
def build(b=B, d=D, r=R):
    """Build the single-core SPMD program (identical on all cores)."""
    KT = d // 128  # k-tiles over feature dim
    QT = KT // 2  # DoubleRow k-pair tiles
    RT = r // 128  # row tiles per core
    NT = b // 128  # row blocks of the norm copy
    GW = min(2048, b)  # PSUM group width (4 banks)
    NG = b // GW
    TPG = GW // 128  # norm row-blocks per group
    LW = min(2048, b)  # DMA piece width for the transposed copy
    NLW = b // LW

    nc = bacc.Bacc(None, target_bir_lowering=False)
    ftd = nc.dram_tensor("features_t", [d, b], BF16, kind="ExternalInput")
    fnd = nc.dram_tensor("features_n", [b, d], BF16, kind="ExternalInput")
    nscr = nc.dram_tensor("nscratch", [b], BF16, kind="Internal")
    outd = nc.dram_tensor("losses", [r], F32, kind="ExternalOutput")

    with tile.TileContext(nc) as tc:
        with (
            tc.tile_pool(name="ftp", bufs=1) as ftp,
            tc.tile_pool(name="sing", bufs=1) as sing,
            tc.tile_pool(name="sq", bufs=6) as sqp,
            tc.tile_pool(name="nrm", bufs=3) as nrmp,
            tc.tile_pool(name="dump", bufs=2) as dumpp,
            tc.tile_pool(name="mm", bufs=2, space="PSUM") as mmp,
        ):
            ft = [
                ftp.tile([128, b], BF16, tag=f"ft{k}", name=f"ft{k}")
                for k in range(KT)
            ]
            ftn = ftp.tile([128, NT * d], BF16, tag="ftn", name="ftn")
            ft8 = [
                ftp.tile([128, 2, b], FP8, tag=f"ft8_{q}", name=f"ft8_{q}")
                for q in range(QT)
            ]

            # Pre-place the one ACT table set that covers every function
            # used below (exp, ln, copy): natural_log_exp_and_others.
            _ln_exp_set = 6
            _tl = mybir.InstLoadActFuncSet(
                name=nc.get_next_instruction_name(),
                act_func_set_id=_ln_exp_set,
                ins=[],
                outs=[],
            )
            nc.scalar.add_instruction(_tl)

            # ---- loads: transposed copy split sync/gpsimd by k-tile,
            # row-major copy on sync, piece-major so early chunks land
            # first ----
            for p in range(NLW):
                ls = slice(p * LW, (p + 1) * LW)
                for k in range(KT):
                    eng = nc.sync if k < KT // 2 else nc.gpsimd
                    eng.dma_start(
                        out=ft[k][:, ls],
                        in_=ftd[k * 128 : (k + 1) * 128, ls],
                    )
                # norm-copy rows for the same local columns: blocks
                # t in [p*LW/128, (p+1)*LW/128)
                t0, t1 = p * LW // 128, (p + 1) * LW // 128
                nc.sync.dma_start(
                    out=ftn[:, t0 * d : t1 * d].rearrange(
                        "p (t f) -> p t f", f=d
                    ),
                    in_=fnd[t0 * 128 : t1 * 128, :].rearrange(
                        "(t p) f -> p t f", p=128
                    ),
                )

            negI = sing.tile([128, 128], BF16)
            nc.gpsimd.memset(negI[:], 0.0)
            nc.gpsimd.affine_select(
                out=negI[:],
                in_=negI[:],
                compare_op=ALU.not_equal,
                fill=-1e30,
                base=0,
                pattern=[[-1, 128]],
                channel_multiplier=1,
            )
            eyeb = sing.tile([128, 128], BF16)
            nc.gpsimd.memset(eyeb[:], 0.0)
            nc.gpsimd.affine_select(
                out=eyeb[:],
                in_=eyeb[:],
                compare_op=ALU.not_equal,
                fill=1.0,
                base=0,
                pattern=[[-1, 128]],
                channel_multiplier=1,
            )
            negTI = sing.tile([128, 128], F32)
            nc.gpsimd.memset(negTI[:], 0.0)
            nc.gpsimd.affine_select(
                out=negTI[:],
                in_=negTI[:],
                compare_op=ALU.not_equal,
                fill=-INV_T / FSCALE,
                base=0,
                pattern=[[-1, 128]],
                channel_multiplier=1,
            )
            negC = sing.tile([128, 1], F32)
            nc.vector.memset(negC[:], -INV_T)
            ln4c = sing.tile([128, 1], F32)
            nc.vector.memset(ln4c[:], LN_FS_HALF)

            ssqN = sing.tile([128, NT], F32)  # ||f||^2, partition-major
            invN = sing.tile([128, NT], BF16)  # 4/||f||, partition-major
            inv_row = sing.tile([1, b], BF16)  # 4/||f|| as a row
            bc_sb = sing.tile([128, b], BF16)  # broadcast of inv_row
            spart = sing.tile([128, RT, NG], F32)
            posn = sing.tile([128, RT], F32)
            ysb = sing.tile([128, RT], F32)
            ssum = sing.tile([128, RT], F32)
            lnS = sing.tile([128, RT], F32)

            def sumsq_span(g):
                """ssq for row-blocks covering group g's columns (DVE)."""
                if g >= NG:
                    return
                for t in range(g * TPG, (g + 1) * TPG):
                    fs = slice(t * d, (t + 1) * d)
                    sq = sqp.tile([128, d], BF16, name=f"sqn{t}", tag="sq")
                    nc.vector.tensor_mul(sq[:], ftn[:, fs], ftn[:, fs])
                    nc.vector.tensor_reduce(
                        out=ssqN[:, t : t + 1],
                        in_=sq[:],
                        axis=mybir.AxisListType.X,
                        op=ALU.add,
                    )

            def norm_span(g):
                """inv = 4/sqrt(ssq) for group g's columns: ACT on the
                [128, TPG] slice, DRAM round-trip to [1, GW], then
                partition_broadcast."""
                if g >= NG:
                    return
                ts = slice(g * TPG, (g + 1) * TPG)
                lns = nrmp.tile([128, TPG], F32, name=f"lns{g}", tag="lns")
                nc.scalar.activation(out=lns[:], in_=ssqN[:, ts], func=AF.Ln)
                nc.scalar.activation(
                    out=invN[:, ts],
                    in_=lns[:],
                    func=AF.Exp,
                    scale=-0.5,
                    bias=ln4c[:],
                )
                cs = slice(g * GW, (g + 1) * GW)
                nc.sync.dma_start(
                    out=nscr[cs].rearrange("(t p) -> p t", p=128),
                    in_=invN[:, ts],
                )
                nc.sync.dma_start(out=inv_row[0:1, cs], in_=nscr[cs])
                nc.gpsimd.partition_broadcast(bc_sb[:, cs], inv_row[0:1, cs])

            def scale_span(g):
                """ft *= bc in place (DVE 2x), then SWDGE cast to fp8."""
                if g >= NG:
                    return
                cs = slice(g * GW, (g + 1) * GW)
                for k in range(KT):
                    nc.vector.tensor_mul(ft[k][:, cs], ft[k][:, cs], bc_sb[:, cs])
                for k in range(KT):
                    nc.gpsimd.dma_start(
                        out=ft8[k // 2][:, k % 2, cs], in_=ft[k][:, cs]
                    )

            sumsq_span(0)
            sumsq_span(1)
            norm_span(0)
            scale_span(0)
            for g in range(NG):
                g0 = g * GW
                norm_span(g + 1)
                scale_span(g + 1)
                for t in range(RT):
                    ps = mmp.tile([128, GW], F32, tag="mm")
                    for q in range(QT):
                        for n2 in range(GW // 512):
                            nc.tensor.matmul(
                                ps[:, n2 * 512 : (n2 + 1) * 512],
                                ft8[q][:, :, t * 128 : (t + 1) * 128],
                                ft8[q][:, :, g0 + n2 * 512 : g0 + (n2 + 1) * 512],
                                start=(q == 0),
                                stop=(q == QT - 1),
                                perf_mode=DR,
                            )
                    pcol = b // 2 + t * 128
                    if g0 <= pcol < g0 + GW:
                        off = pcol - g0
                        dmp = dumpp.tile([128, 128], F32)
                        nc.vector.tensor_mul(dmp[:], ps[:, off : off + 128], negTI[:])
                        nc.vector.tensor_reduce(
                            out=posn[:, t : t + 1],
                            in_=dmp[:],
                            axis=mybir.AxisListType.X,
                            op=ALU.add,
                        )
                    dcol = t * 128
                    if g0 <= dcol < g0 + GW:
                        off = dcol - g0
                        nc.tensor.matmul(
                            ps[:, off : off + 128],
                            eyeb[:],
                            negI[:],
                            start=False,
                            stop=True,
                            skip_group_check=True,
                        )
                    nc.scalar.activation(
                        out=ps[:],
                        in_=ps[:],
                        func=AF.Exp,
                        scale=INV_T / FSCALE,
                        bias=negC[:],
                        accum_out=spart[:, t, g : g + 1],
                    )
                sumsq_span(g + 2)

            # ---- epilogue: y = ln(S) - pos/T ----
            for t in range(RT):
                nc.vector.tensor_reduce(
                    out=ssum[:, t : t + 1],
                    in_=spart[:, t, :],
                    axis=mybir.AxisListType.X,
                    op=ALU.add,
                )
            nc.scalar.activation(out=lnS[:, :RT], in_=ssum[:, :RT], func=AF.Ln)
            nc.vector.tensor_add(ysb[:, :RT], lnS[:, :RT], posn[:, :RT])
            nc.sync.dma_start(
                out=outd[:].rearrange("(t p) -> p t", p=128), in_=ysb[:, :RT]
            )

    nc.finalize()
    return nc


def run(features, b=B, d=D, ncores=NCORES, **kwargs):
    """Run the SPMD kernel; returns (losses[b] fp32, BassKernelResults)."""
    r = b // ncores
    nc = build(b, d, r)
    feats = np.ascontiguousarray(np.asarray(features, dtype=np.float32))
    in_maps = []
    for c in range(ncores):
        rot = np.roll(feats, -c * r, axis=0)
        in_maps.append(
            {
                "features_t": np.ascontiguousarray(rot.T).astype(ml_dtypes.bfloat16),
                "features_n": np.ascontiguousarray(rot).astype(ml_dtypes.bfloat16),
            }
        )
    res = run_bass_kernel_spmd(nc, in_maps, core_ids=list(range(ncores)), **kwargs)
    y = np.concatenate([res.results[c]["losses"] for c in range(ncores)])
    return y, res


def kernel(features):
    y, _ = run(features)
    loss = INV_T + float(np.mean(y.astype(np.float64)))
    return np.float32(loss)


# revision 10
# speedup vs baseline: 1.0473x; 1.0473x over previous
"""Trainium2 Bass kernel for SimCLR-style contrastive loss (B=8192, D=512).

Math (matches reference):
    f_norm = f / ||f||
    sim    = f_norm @ f_norm.T / T
    lse_i  = logsumexp_{j != i} sim_ij
    pos_i  = sim[i, (i + B/2) mod B]
    loss   = mean_i(lse_i - pos_i)

Device strategy (8 cores, data parallel over rows):
  - Host passes each core a row-rotated transposed bf16 copy [D, B]
    (for the gram matmul) plus the same rotation row-major [B, D]
    (for norms), so one SPMD program works for every core: core c's
    rotated copy starts at global row c*R, its own rows occupy local
    columns [0, R), and row r's positive partner sits at local column
    r + B/2.
  - Norms from the row-major copy: square + free-axis reduce on DVE
    (all bf16 SBUF, 2x mode) giving ssq in [128, B/128] partition-major
    layout, so the ln -> exp(-0.5*ln + ln 4) chain on ACT costs ~64
    cycles per 16 columns instead of a [1, B] pass.  The x16 on the
    gram keeps fp8 features clear of the e4m3 subnormal range and is
    compensated in the final exp scale.  inv goes through a DRAM
    round-trip to reshape [128, B/128] -> [1, B], then GpSimd
    partition_broadcast -> [128, B].
  - Features scaled in place on DVE (bf16 2x), then cast bf16 -> fp8e4
    by gpsimd-issued SWDGE DMAs (the only engine whose DMA casts)
    into DoubleRow layout [128, 2, B].
  - Main gram matmul in fp8e4 MatmulPerfMode.DoubleRow: one
    instruction contracts K=256, so D=512 streams each output column
    only twice.  PSUM groups are 2048 wide (2 bufs x 4 banks = all 8,
    norms no longer need PSUM) to amortize ACT per-instruction cost.
  - Each [128 x 2048] PSUM group gets: diagonal killed with a -1e30*I
    accumulating bf16 matmul, partner diagonal extracted via
    tensor_mul against (-1/(16T))*I + row reduce, then fused
    exp+row-sum on ACT (exp((16cos)/(16T) - 1/T), in place in PSUM).
  - Per-row output y_i = ln(sum_i) - pos_cos_i/T returned as [R] f32;
    host computes loss = 1/T + mean(y).
"""

import functools
import sys

sys.path.insert(0, "/opt/trn_rl_repo")

import ml_dtypes
import numpy as np

import concourse.bass as bass
import concourse.mybir as mybir
import concourse.tile as tile
from concourse import bacc
from concourse.bass_utils import run_bass_kernel_spmd
from concourse.tile import add_dep_helper

B = 8192
D = 512
NCORES = 8
R = B // NCORES  # rows per core
TEMP = 0.07
INV_T = 1.0 / TEMP
FSCALE = 16.0  # features carry 4/norm each -> gram = 16*cos
LN_FS_HALF = float(np.log(4.0))

F32 = mybir.dt.float32
BF16 = mybir.dt.bfloat16
FP8 = mybir.dt.float8e4
AF = mybir.ActivationFunctionType
ALU = mybir.AluOpType
DR = mybir.MatmulPerfMode.DoubleRow


@functools.lru_cache(maxsize=None)
def build(b=B, d=D, r=R):
    """Build the single-core SPMD program (identical on all cores)."""
    KT = d // 128  # k-tiles over feature dim
    QT = KT // 2  # DoubleRow k-pair tiles
    RT = r // 128  # row tiles per core
    NT = b // 128  # row blocks of the norm copy
    GW = min(2048, b)  # PSUM group width (4 banks)
    NG = b // GW
    TPG = GW // 128  # norm row-blocks per group
    LW = min(2048, b)  # DMA piece width for the transposed copy
    NLW = b // LW

    nc = bacc.Bacc(None, target_bir_lowering=False)
    ftd = nc.dram_tensor("features_t", [d, b], BF16, kind="ExternalInput")
    fnd = nc.dram_tensor("features_n", [b, d], BF16, kind="ExternalInput")
    nscr = nc.dram_tensor("nscratch", [b], BF16, kind="Internal")
    outd = nc.dram_tensor("losses", [r], F32, kind="ExternalOutput")

    with tile.TileContext(nc) as tc:
        with (
            tc.tile_pool(name="ftp", bufs=1) as ftp,
            tc.tile_pool(name="sing", bufs=1) as sing,
            tc.tile_pool(name="sq", bufs=6) as sqp,
            tc.tile_pool(name="nrm", bufs=3) as nrmp,
            tc.tile_pool(name="dump", bufs=2) as dumpp,
            tc.tile_pool(name="mm", bufs=2, space="PSUM") as mmp,
        ):
            ft = [
                ftp.tile([128, b], BF16, tag=f"ft{k}", name=f"ft{k}")
                for k in range(KT)
            ]
            ftn = ftp.tile([128, NT * d], BF16, tag="ftn", name="ftn")
            ft8 = [
                ftp.tile([128, 2, b], FP8, tag=f"ft8_{q}", name=f"ft8_{q}")
                for q in range(QT)
            ]

            # Pre-place the one ACT table set that covers every function
            # used below (exp, ln, copy): natural_log_exp_and_others.
            _ln_exp_set = 6
            _tl = mybir.InstLoadActFuncSet(
                name=nc.get_next_instruction_name(),
                act_func_set_id=_ln_exp_set,
                ins=[],
                outs=[],
            )
            nc.scalar.add_instruction(_tl)

            # ---- loads: transposed copy split sync/gpsimd by k-tile,
            # row-major copy on sync, piece-major so early chunks land
            # first ----
            for p in range(NLW):
                ls = slice(p * LW, (p + 1) * LW)
                # norm-copy rows for this column span first (ssq leads
                # the pipeline), then the transposed copy: k 0-1 on the
                # sync queue, k 2-3 on the DVE-issued queue.  The gpsimd
                # SWDGE queue is left entirely to the fp8 casts and the
                # ACT queue to the norm round-trips, so neither waits
                # behind megabytes of input loads.
                t0, t1 = p * LW // 128, (p + 1) * LW // 128
                nc.sync.dma_start(
                    out=ftn[:, t0 * d : t1 * d].rearrange(
                        "p (t f) -> p t f", f=d
                    ),
                    in_=fnd[t0 * 128 : t1 * 128, :].rearrange(
                        "(t p) f -> p t f", p=128
                    ),
                )
                for k in range(KT):
                    eng = nc.sync if k < KT // 2 else nc.scalar
                    eng.dma_start(
                        out=ft[k][:, ls],
                        in_=ftd[k * 128 : (k + 1) * 128, ls],
                    )

            negI = sing.tile([128, 128], BF16)
            nc.gpsimd.memset(negI[:], 0.0)
            nc.gpsimd.affine_select(
                out=negI[:],
                in_=negI[:],
                compare_op=ALU.not_equal,
                fill=-1e30,
                base=0,
                pattern=[[-1, 128]],
                channel_multiplier=1,
            )
            eyeb = sing.tile([128, 128], BF16)
            nc.gpsimd.memset(eyeb[:], 0.0)
            nc.gpsimd.affine_select(
                out=eyeb[:],
                in_=eyeb[:],
                compare_op=ALU.not_equal,
                fill=1.0,
                base=0,
                pattern=[[-1, 128]],
                channel_multiplier=1,
            )
            negTI = sing.tile([128, 128], F32)
            nc.gpsimd.memset(negTI[:], 0.0)
            nc.gpsimd.affine_select(
                out=negTI[:],
                in_=negTI[:],
                compare_op=ALU.not_equal,
                fill=-INV_T / FSCALE,
                base=0,
                pattern=[[-1, 128]],
                channel_multiplier=1,
            )
            negC = sing.tile([128, 1], F32)
            nc.vector.memset(negC[:], -INV_T)
            ln4c = sing.tile([128, 1], F32)
            nc.vector.memset(ln4c[:], LN_FS_HALF)

            ssqN = sing.tile([128, NT], F32)  # ||f||^2, partition-major
            invN = sing.tile([128, NT], BF16)  # 4/||f||, partition-major
            inv_row = sing.tile([1, b], BF16)  # 4/||f|| as a row
            bc_sb = sing.tile([128, b], BF16)  # broadcast of inv_row
            spart = sing.tile([128, RT, NG], F32)
            posn = sing.tile([128, RT], F32)
            ysb = sing.tile([128, RT], F32)
            ssum = sing.tile([128, RT], F32)
            lnS = sing.tile([128, RT], F32)

            def sumsq_span(g):
                """ssq for row-blocks covering group g's columns (DVE)."""
                if g >= NG:
                    return
                for t in range(g * TPG, (g + 1) * TPG):
                    fs = slice(t * d, (t + 1) * d)
                    sq = sqp.tile([128, d], BF16, name=f"sqn{t}", tag="sq")
                    nc.vector.tensor_mul(sq[:], ftn[:, fs], ftn[:, fs])
                    nc.vector.tensor_reduce(
                        out=ssqN[:, t : t + 1],
                        in_=sq[:],
                        axis=mybir.AxisListType.X,
                        op=ALU.add,
                    )

            def norm_span(g):
                """inv = 4/sqrt(ssq) for group g's columns: ACT on the
                [128, TPG] slice, DRAM round-trip to [1, GW], then
                partition_broadcast."""
                if g >= NG:
                    return
                ts = slice(g * TPG, (g + 1) * TPG)
                lns = nrmp.tile([128, TPG], F32, name=f"lns{g}", tag="lns")
                nc.scalar.activation(out=lns[:], in_=ssqN[:, ts], func=AF.Ln)
                nc.scalar.activation(
                    out=invN[:, ts],
                    in_=lns[:],
                    func=AF.Exp,
                    scale=-0.5,
                    bias=ln4c[:],
                )
                cs = slice(g * GW, (g + 1) * GW)
                nc.gpsimd.dma_start(
                    out=nscr[cs].rearrange("(t p) -> p t", p=128),
                    in_=invN[:, ts],
                )
                nc.gpsimd.dma_start(out=inv_row[0:1, cs], in_=nscr[cs])
                nc.gpsimd.partition_broadcast(bc_sb[:, cs], inv_row[0:1, cs])

            def scale_span(g):
                """ft *= bc in place (DVE 2x), then SWDGE cast to fp8."""
                if g >= NG:
                    return
                cs = slice(g * GW, (g + 1) * GW)
                for k in range(KT):
                    nc.vector.tensor_mul(ft[k][:, cs], ft[k][:, cs], bc_sb[:, cs])
                for k in range(KT):
                    nc.gpsimd.dma_start(
                        out=ft8[k // 2][:, k % 2, cs], in_=ft[k][:, cs]
                    )

            sumsq_span(0)
            sumsq_span(1)
            norm_span(0)
            scale_span(0)
            for g in range(NG):
                g0 = g * GW
                norm_span(g + 1)
                scale_span(g + 1)
                for t in range(RT):
                    ps = mmp.tile([128, GW], F32, tag="mm")
                    for q in range(QT):
                        for n2 in range(GW // 512):
                            nc.tensor.matmul(
                                ps[:, n2 * 512 : (n2 + 1) * 512],
                                ft8[q][:, :, t * 128 : (t + 1) * 128],
                                ft8[q][:, :, g0 + n2 * 512 : g0 + (n2 + 1) * 512],
                                start=(q == 0),
                                stop=(q == QT - 1),
                                perf_mode=DR,
                            )
                    pcol = b // 2 + t * 128
                    if g0 <= pcol < g0 + GW:
                        off = pcol - g0
                        dmp = dumpp.tile([128, 128], F32)
                        nc.vector.tensor_mul(dmp[:], ps[:, off : off + 128], negTI[:])
                        nc.vector.tensor_reduce(
                            out=posn[:, t : t + 1],
                            in_=dmp[:],
                            axis=mybir.AxisListType.X,
                            op=ALU.add,
                        )
                    dcol = t * 128
                    if g0 <= dcol < g0 + GW:
                        off = dcol - g0
                        nc.tensor.matmul(
                            ps[:, off : off + 128],
                            eyeb[:],
                            negI[:],
                            start=False,
                            stop=True,
                            skip_group_check=True,
                        )
                    nc.scalar.activation(
                        out=ps[:],
                        in_=ps[:],
                        func=AF.Exp,
                        scale=INV_T / FSCALE,
                        bias=negC[:],
                        accum_out=spart[:, t, g : g + 1],
                    )
                sumsq_span(g + 2)

            # ---- epilogue: y = ln(S) - pos/T ----
            for t in range(RT):
                nc.vector.tensor_reduce(
                    out=ssum[:, t : t + 1],
                    in_=spart[:, t, :],
                    axis=mybir.AxisListType.X,
                    op=ALU.add,
                )
            nc.scalar.activation(out=lnS[:, :RT], in_=ssum[:, :RT], func=AF.Ln)
            nc.vector.tensor_add(ysb[:, :RT], lnS[:, :RT], posn[:, :RT])
            nc.sync.dma_start(
                out=outd[:].rearrange("(t p) -> p t", p=128), in_=ysb[:, :RT]
            )

    nc.finalize()
    return nc


def run(features, b=B, d=D, ncores=NCORES, **kwargs):
    """Run the SPMD kernel; returns (losses[b] fp32, BassKernelResults)."""
    r = b // ncores
    nc = build(b, d, r)
    feats = np.ascontiguousarray(np.asarray(features, dtype=np.float32))
    in_maps = []
    for c in range(ncores):
        rot = np.roll(feats, -c * r, axis=0)
        in_maps.append(
            {
                "features_t": np.ascontiguousarray(rot.T).astype(ml_dtypes.bfloat16),
                "features_n": np.ascontiguousarray(rot).astype(ml_dtypes.bfloat16),
            }
        )
    res = run_bass_kernel_spmd(nc, in_maps, core_ids=list(range(ncores)), **kwargs)
    y = np.concatenate([res.results[c]["losses"] for c in range(ncores)])
    return y, res


def kernel(features):
    y, _ = run(features)
    loss = INV_T + float(np.mean(y.astype(np.float64)))
    return np.float32(loss)
